# revision 16
# baseline (speedup 1.0000x reference)
"""Causal self-attention (B=2, T=2048, C=1024, H=16) on 8 TRN2 NeuronCores.

Sharding: 8 cores = 2 batches x 4 head-groups (4 heads each).
Each core computes qkv projection for its heads, attention, and a partial
output projection (its rows of w_proj); the host sums the 4 partials per
batch and adds b_proj.

Device-side design (v2 — software-pipelined):
  - Emission interleaves four streams so no engine idles at phase
    boundaries: projection chunk tc+1 and output-projection chunk ici-1
    are emitted as PE "filler" items inside attention chunk ici (which is
    ACT-exp bound).
  - RoPE: head dims are reordered PAIR-ADJACENT on partitions (host-side
    weight/cos/sin permutation), so rotate-half is a single full-rate DVE
    stream_shuffle; the sin-product runs on GpSimd; only 2 tensor_tensor
    passes remain on the DVE per tile.
  - Causal trim: for diagonal j-blocks only i >= jb*128 columns are
    computed (QK, exp, AV all shrink); the remaining triangle is masked
    by a fixed [128,128] affine_select per diagonal half.
  - Scores are computed transposed (S^T[j,i]) so softmax needs no
    partition reductions: AV with a ones-augmented V (65th row) yields
    denominators, which are broadcast via a tiny K=1 PE matmul and
    inverted with reciprocal_approx_fast.
"""

import sys
import os
from collections import deque

for _p in ("/opt/trn_rl_repo", "/root/.axon_site/_ro/trn_rl_repo"):
    if os.path.isdir(_p) and _p not in sys.path:
        sys.path.insert(0, _p)

import numpy as np
import concourse.bass as bass
import concourse.mybir as mybir
import concourse.tile as tile
from concourse import bacc
from concourse.bass_utils import run_bass_kernel_spmd

B, T, C, H = 2, 2048, 1024, 16
HS = C // H          # 64
HALF = HS // 2       # 32
NCORES = 8
NH = 4               # heads per core
TCH = 512            # t-chunk for projections / i-chunk for attention
CB = C // 128        # 8 contraction blocks
NTB = T // 128       # 16 t/j blocks
NCH = T // TCH       # 4 chunks
F32 = mybir.dt.float32
F32R = mybir.dt.float32r
BF16 = mybir.dt.bfloat16
USE_BF16 = os.environ.get("KERNEL_BF16", "0") == "1"
MMD = BF16 if USE_BF16 else F32R
AF = mybir.ActivationFunctionType
ALU = mybir.AluOpType
SWAP_MASK = [i ^ 1 for i in range(32)]   # pairwise partner within 32-block

_CACHED = {}


def _half_geom(ici, jb):
    """(i-offset within the 512 chunk, width) for j-block jb of chunk ici."""
    r = jb - 4 * ici
    off = max(r, 0) * 128
    return off, TCH - off


def _build_nc():
    nc = bacc.Bacc("TRN2", target_bir_lowering=False, debug=False)

    xt = nc.dram_tensor("xt", [C, T], MMD, kind="ExternalInput").ap()
    wqk = nc.dram_tensor("wqk", [C, 512], MMD, kind="ExternalInput").ap()
    wv = nc.dram_tensor("wv", [C, 256], MMD, kind="ExternalInput").ap()
    wproj = nc.dram_tensor("wproj", [256, C], MMD, kind="ExternalInput").ap()
    bqk = nc.dram_tensor("bqk", [4, 128], F32, kind="ExternalInput").ap()
    cosrep = nc.dram_tensor("cosrep", [128, T], F32, kind="ExternalInput").ap()
    sinsw = nc.dram_tensor("sinsw", [128, T], F32, kind="ExternalInput").ap()
    one_row = nc.dram_tensor("one_row", [1, 128], F32R, kind="ExternalInput").ap()
    yout = nc.dram_tensor("yout", [T, C], F32, kind="ExternalOutput").ap()

    with tile.TileContext(nc) as tc:
        with (
            tc.tile_pool(name="const", bufs=1) as const,
            tc.tile_pool(name="persist", bufs=1) as persist,
            tc.tile_pool(name="work", bufs=2) as work,
            tc.tile_pool(name="xtp", bufs=2) as xtp,
            tc.tile_pool(name="attnp", bufs=4) as attnp,
            tc.tile_pool(name="ps", bufs=1, space="PSUM") as ps,
        ):
            # ---- constants -------------------------------------------------
            # DMA issue placement matters: sync carries the startup-critical
            # stream; scalar gets the big weight bodies (it is idle pre-rope);
            # cos/sin are split per-chunk so rope chunk 0 starts early.
            wqk_sb = const.tile([128, CB * 512], MMD)
            nc.sync.dma_start(out=wqk_sb[:, 0:512], in_=wqk[0:128, :])
            xts = {}
            cos_sb = const.tile([128, T], F32)
            sin_sb = const.tile([128, T], F32)

            def load_xt(tcid):
                if tcid >= NCH:
                    return
                t = xtp.tile([128, CB * TCH], MMD, tag="xt", bufs=2, name=f"xt{tcid}")
                tv = t.rearrange("p (cb t) -> p cb t", cb=CB)
                src = xt.rearrange("(cb p) t -> p cb t", p=128)[
                    :, :, tcid * TCH : (tcid + 1) * TCH
                ]
                if tcid == 0:
                    nc.sync.dma_start(out=tv[:, 0:2], in_=src[:, 0:2])
                else:
                    nc.sync.dma_start(out=tv, in_=src)
                xts[tcid] = t

            load_xt(0)
            # sync queue continues in PE-dependency order
            nc.sync.dma_start(
                out=wqk_sb[:, 512:].rearrange("p (cb m) -> p cb m", cb=CB - 1),
                in_=wqk[128:, :].rearrange("(cb p) m -> p cb m", p=128),
            )
            nc.sync.dma_start(
                out=xts[0].rearrange("p (cb t) -> p cb t", cb=CB)[:, 2:],
                in_=xt.rearrange("(cb p) t -> p cb t", p=128)[:, 2:, 0:TCH],
            )
            # scalar queue: rope constants + weight bodies needed from ~5us on
            bqk_sb = const.tile([128, 4], F32)
            for mt in range(4):
                nc.scalar.dma_start(out=bqk_sb[:, mt : mt + 1], in_=bqk[mt, :][:, None])
            ones1 = const.tile([1, 128], F32R)
            nc.scalar.dma_start(out=ones1, in_=one_row)
            nc.scalar.dma_start(out=cos_sb[:, 0:TCH], in_=cosrep[:, 0:TCH])
            nc.scalar.dma_start(out=sin_sb[:, 0:TCH], in_=sinsw[:, 0:TCH])
            wv_sb = const.tile([128, CB * 256], MMD)
            nc.scalar.dma_start(
                out=wv_sb.rearrange("p (cb m) -> p cb m", cb=CB),
                in_=wv.rearrange("(cb p) m -> p cb m", p=128),
            )
            nc.scalar.dma_start(out=cos_sb[:, TCH:], in_=cosrep[:, TCH:])
            nc.scalar.dma_start(out=sin_sb[:, TCH:], in_=sinsw[:, TCH:])
            wproj_sb = const.tile([128, 2 * C], MMD)
            scr = const.tile([128, 1], F32)
            nc.gpsimd.memset(scr, 0.0)
            # warm the exp table set while DMAs run
            nc.scalar.activation(scr, scr, AF.Exp, scale=1.0)

            # ---- persistent intermediates ----------------------------------
            qt_sb = persist.tile([128, 2 * T], MMD)   # [Q01 | Q23], pair-adj d
            kt_sb = persist.tile([128, 2 * T], MMD)
            v_sb = persist.tile([128, NTB * 260], MMD)  # per jb: 4x(64 v + 1 one)
            ctx0 = persist.tile([128, T], MMD)        # heads 0,1 ctxT (normalized)
            ctx1 = persist.tile([128, T], MMD)
            ones_sb = const.tile([128, 64], F32)
            nc.gpsimd.memset(ones_sb, 1.0)
            nc.vector.tensor_copy(
                v_sb.rearrange("p (tb h d) -> p tb h d", tb=NTB, h=4)[:, :, :, 64:65],
                ones_sb.rearrange("p (a b c) -> p a b c", a=NTB, b=4),
            )

            # ---- projection chunk items ------------------------------------
            def qk_item(tcid, mt):
                def go():
                    xt_t = xts[tcid]
                    pq = ps.tile([128, 512], F32, tag="pa", bufs=2, name=f"pq{tcid}_{mt}")
                    for cb in range(CB):
                        nc.tensor.matmul(
                            pq,
                            lhsT=wqk_sb[:, cb * 512 + mt * 128 : cb * 512 + (mt + 1) * 128],
                            rhs=xt_t[:, cb * TCH : (cb + 1) * TCH],
                            start=(cb == 0),
                            stop=(cb == CB - 1),
                        )
                    qb = work.tile([128, 512], F32, tag="qb", bufs=2, name=f"qb{tcid}_{mt}")
                    nc.scalar.activation(
                        qb, pq, AF.Identity, bias=bqk_sb[:, mt : mt + 1], scale=1.0
                    )
                    qsw = work.tile([128, 512], F32, tag="qsw", bufs=2, name=f"qsw{tcid}_{mt}")
                    nc.vector.stream_shuffle(qsw, qb, SWAP_MASK)
                    tsl = slice(tcid * TCH, (tcid + 1) * TCH)
                    t1 = work.tile([128, 512], F32, tag="t1", bufs=2, name=f"t1_{tcid}_{mt}")
                    nc.vector.tensor_mul(t1, qsw, sin_sb[:, tsl])
                    m1 = work.tile([128, 512], F32, tag="m1", bufs=2, name=f"m1_{tcid}_{mt}")
                    nc.vector.tensor_mul(m1, qb, cos_sb[:, tsl])
                    dest = qt_sb if mt < 2 else kt_sb
                    dcol = (mt % 2) * T + tcid * TCH
                    nc.vector.tensor_add(dest[:, dcol : dcol + 512], m1, t1)
                return go

            def v_item(tcid, half):
                def go():
                    xt_t = xts[tcid]
                    pv = ps.tile([128, 512], F32, tag="pa", bufs=2, name=f"pv{tcid}_{half}")
                    for sub in range(2):
                        tl = half * 2 + sub
                        for cb in range(CB):
                            nc.tensor.matmul(
                                pv[:, sub * 256 : (sub + 1) * 256],
                                lhsT=xt_t[:, cb * TCH + tl * 128 : cb * TCH + (tl + 1) * 128],
                                rhs=wv_sb[:, cb * 256 : (cb + 1) * 256],
                                start=(cb == 0),
                                stop=(cb == CB - 1),
                            )
                    tb0 = tcid * 4 + half * 2
                    nc.scalar.copy(
                        v_sb[:, tb0 * 260 : (tb0 + 2) * 260].rearrange(
                            "p (tb h d) -> p tb h d", tb=2, h=4
                        )[:, :, :, 0:64],
                        pv.rearrange("p (tb h d) -> p tb h d", tb=2, h=4),
                    )
                return go

            def proj_items(tcid):
                # mt order 0,2,1,3: pair0's q/k rope completes first so the
                # next attention chunk's first duos unblock earlier
                items = [lambda t=tcid: load_xt(t + 1)]
                items += [qk_item(tcid, mt) for mt in (0, 2, 1, 3)]
                items += [v_item(tcid, h) for h in (0, 1)]
                return items

            # ---- output-projection items (one psum bank each) ---------------
            def out_item(ici, tl, ncol):
                def go():
                    tb = ici * 4 + tl
                    yp = ps.tile([128, 512], F32, tag="pa", bufs=2, name=f"yp{tb}_{ncol}")
                    for cb in range(2):
                        ctx_t = ctx0 if cb == 0 else ctx1
                        nc.tensor.matmul(
                            yp,
                            lhsT=ctx_t[:, tb * 128 : (tb + 1) * 128],
                            rhs=wproj_sb[:, cb * C + ncol * 512 : cb * C + (ncol + 1) * 512],
                            start=(cb == 0),
                            stop=(cb == 1),
                        )
                    ysb = work.tile([128, 512], F32, tag="ysb", bufs=2, name=f"ysb{tb}_{ncol}")
                    nc.vector.tensor_copy(ysb, yp)
                    nc.sync.dma_start(
                        out=yout[tb * 128 : (tb + 1) * 128, ncol * 512 : (ncol + 1) * 512],
                        in_=ysb,
                    )
                return go

            def out_items(ici):
                return [out_item(ici, tl, ncol) for tl in range(4) for ncol in range(2)]

            # ---- attention chunk -------------------------------------------
            def attn_chunk(ici, fillers):
                njb = 4 * (ici + 1)
                i0 = ici * TCH
                for pair in range(2):
                    qt_p = qt_sb[:, pair * T : (pair + 1) * T]
                    kt_p = kt_sb[:, pair * T : (pair + 1) * T]
                    ctxps = ps.tile([65, 1024], F32, tag="ctx", bufs=1, name=f"cps{ici}_{pair}")
                    pending = []

                    def emit_av(entry):
                        ats, geom = entry
                        for hh in range(2):
                            for (jb, off, w, pk) in geom:
                                h_loc = pair * 2 + hh
                                nc.tensor.matmul(
                                    ctxps[0:65, hh * 512 + off : hh * 512 + off + w],
                                    lhsT=v_sb[:, jb * 260 + h_loc * 65 : jb * 260 + (h_loc + 1) * 65],
                                    rhs=ats[hh][:, pk : pk + w],
                                    start=(jb == 0),
                                    stop=(jb == njb - 1),
                                    skip_group_check=True,
                                )

                    for d in range(njb // 2):
                        if fillers:
                            fillers.popleft()()
                        geom = []
                        pk = 0
                        for jb in (2 * d, 2 * d + 1):
                            off, w = _half_geom(ici, jb)
                            geom.append((jb, off, w, pk))
                            pk += w
                        atw = pk
                        sts = [
                            ps.tile([128, 1024], F32, tag="st", bufs=2, name=f"st{ici}_{pair}_{d}_{hh}")
                            for hh in range(2)
                        ]
                        for (jb, off, w, pkh) in geom:
                            for hh in range(2):
                                nc.tensor.matmul(
                                    sts[hh][:, pkh : pkh + w],
                                    lhsT=kt_p[hh * 64 : (hh + 1) * 64, jb * 128 : (jb + 1) * 128],
                                    rhs=qt_p[hh * 64 : (hh + 1) * 64, i0 + off : i0 + off + w],
                                    start=True,
                                    stop=True,
                                )
                        ats = []
                        for hh in range(2):
                            at = attnp.tile([128, 1024], MMD, tag="attn", bufs=4, name=f"at{ici}_{pair}_{d}_{hh}")
                            nc.scalar.activation(at[:, 0:atw], sts[hh][:, 0:atw], AF.Exp, scale=0.125)
                            for (jb, off, w, pkh) in geom:
                                if jb >= 4 * ici:  # diagonal half: triangle in first 128 cols
                                    nc.gpsimd.affine_select(
                                        out=at[:, pkh : pkh + 128],
                                        in_=at[:, pkh : pkh + 128],
                                        compare_op=ALU.is_ge,
                                        fill=0.0,
                                        base=0,
                                        channel_multiplier=-1,
                                        pattern=[[1, 128]],
                                    )
                            ats.append(at)
                        if pending:
                            emit_av(pending.pop(0))
                        pending.append((ats, geom))
                    while pending:
                        emit_av(pending.pop(0))

                    # normalization: denom row 64 -> PE broadcast -> 1/x -> mul
                    if fillers:
                        fillers.popleft()()
                    dn = work.tile([1, 1024], F32R, tag="dn", bufs=2, name=f"dn{ici}_{pair}")
                    nc.vector.tensor_copy(dn, ctxps[64:65, :])
                    bc = ps.tile([128, 1024], F32, tag="st", bufs=2, name=f"bc{ici}_{pair}")
                    for s in (0, 512):
                        nc.tensor.matmul(
                            bc[:, s : s + 512],
                            lhsT=ones1,
                            rhs=dn[:, s : s + 512],
                            start=True,
                            stop=True,
                        )
                    inv = work.tile([128, 1024], F32, tag="inv", bufs=2, name=f"inv{ici}_{pair}")
                    nc.vector.reciprocal_approx_fast(out=inv, in_=bc)
                    ctx_p = ctx0 if pair == 0 else ctx1
                    for hh in range(2):
                        nc.vector.tensor_mul(
                            ctx_p[hh * 64 : (hh + 1) * 64, i0 : i0 + 512],
                            ctxps[0:64, hh * 512 : (hh + 1) * 512],
                            inv[hh * 64 : (hh + 1) * 64, hh * 512 : (hh + 1) * 512],
                        )

            # ---- schedule ---------------------------------------------------
            for f in proj_items(0):
                f()
            nc.scalar.dma_start(
                out=wproj_sb.rearrange("p (cb n) -> p cb n", cb=2),
                in_=wproj.rearrange("(cb p) n -> p cb n", p=128),
            )
            fillers = deque(proj_items(1))
            attn_chunk(0, fillers)
            while fillers:
                fillers.popleft()()
            fillers = deque(proj_items(2))
            attn_chunk(1, fillers)
            while fillers:
                fillers.popleft()()
            fillers = deque(proj_items(3) + out_items(0))
            attn_chunk(2, fillers)
            while fillers:
                fillers.popleft()()
            fillers = deque(out_items(1) + out_items(2))
            attn_chunk(3, fillers)
            while fillers:
                fillers.popleft()()
            for f in out_items(3):
                f()

    nc.compile()
    return nc


def _prep_core_inputs(x, cos, sin, w_attn, b_attn, w_proj):
    """Build the 8 per-core input maps (host-side shard/reorder)."""
    import ml_dtypes

    mmnp = ml_dtypes.bfloat16 if USE_BF16 else np.float32
    x = np.asarray(x, dtype=np.float32)
    cos = np.asarray(cos, dtype=np.float32).reshape(T, HALF)
    sin = np.asarray(sin, dtype=np.float32).reshape(T, HALF)
    w_attn = np.asarray(w_attn, dtype=np.float32)
    b_attn = np.asarray(b_attn, dtype=np.float32)
    w_proj = np.asarray(w_proj, dtype=np.float32)

    cosT = np.ascontiguousarray(cos.T)               # [32, T]
    sinT = np.ascontiguousarray(sin.T)

    # pair-adjacent d order: partition p (within a head's 64) -> orig d
    p64 = np.arange(64)
    d_of_p = (p64 % 2) * HALF + p64 // 2             # [64]
    pairidx = p64 // 2
    member = p64 % 2
    cos64 = cosT[pairidx]                            # [64, T]
    sin64 = sinT[pairidx] * np.where(member == 0, -1.0, 1.0)[:, None].astype(np.float32)
    cosrep = np.tile(cos64, (2, 1))                  # [128, T]
    sin_sw = np.tile(sin64, (2, 1))

    xts = [np.ascontiguousarray(x[b].T).astype(mmnp) for b in range(B)]  # [C, T]

    in_maps = []
    for core in range(NCORES):
        b = core // 4
        g = core % 4
        heads = [4 * g + i for i in range(NH)]
        # q/k M-tiles: [Q(h0,h1), Q(h2,h3), K(h0,h1), K(h2,h3)], pair-adj cols
        qcols, bq = [], []
        for mt, (base, hs) in enumerate(
            [(0, heads[0:2]), (0, heads[2:4]), (C, heads[0:2]), (C, heads[2:4])]
        ):
            cols = np.concatenate([base + h * HS + d_of_p for h in hs])
            qcols.append(cols)
            bq.append(b_attn[cols])
        wqk_c = np.ascontiguousarray(w_attn[:, np.concatenate(qcols)]).astype(mmnp)
        bqk_c = np.stack(bq)                                            # [4, 128]
        vcols = np.concatenate(
            [np.arange(2 * C + h * HS, 2 * C + (h + 1) * HS) for h in heads]
        )
        wv_c = np.ascontiguousarray(w_attn[:, vcols]).astype(mmnp)
        wproj_c = np.ascontiguousarray(w_proj[g * 256 : (g + 1) * 256, :]).astype(mmnp)
        in_maps.append(
            {
                "xt": xts[b],
                "wqk": wqk_c,
                "wv": wv_c,
                "wproj": wproj_c,
                "bqk": np.ascontiguousarray(bqk_c),
                "cosrep": np.ascontiguousarray(cosrep),
                "sinsw": np.ascontiguousarray(sin_sw),
                "one_row": np.ones((1, 128), np.float32),
            }
        )
    return in_maps


def kernel(x, cos, sin, w_attn, b_attn, w_proj, b_proj, _want_trace=False):
    if "nc" not in _CACHED:
        _CACHED["nc"] = _build_nc()
    nc = _CACHED["nc"]
    in_maps = _prep_core_inputs(x, cos, sin, w_attn, b_attn, w_proj)
    res = run_bass_kernel_spmd(
        nc, in_maps, core_ids=list(range(NCORES)), trace=_want_trace
    )
    _CACHED["last_result"] = res
    b_proj = np.asarray(b_proj, dtype=np.float32)
    # v-bias folds out of attention (softmax rows sum to 1): it contributes a
    # constant b_v @ w_proj to every output row, added here with b_proj.
    bv = np.asarray(b_attn, dtype=np.float32)[2 * C : 3 * C]
    bias_full = b_proj + bv @ np.asarray(w_proj, dtype=np.float32)
    out = np.empty((B, T, C), np.float32)
    for b in range(B):
        acc = res.results[b * 4]["yout"].astype(np.float32).copy()
        for g in range(1, 4):
            acc += res.results[b * 4 + g]["yout"]
        out[b] = acc + bias_full[None, :]
    return out


# revision 19
# speedup vs baseline: 1.0952x; 1.0952x over previous
"""Causal self-attention (B=2, T=2048, C=1024, H=16) on 8 TRN2 NeuronCores.

Sharding: 8 cores = 2 batches x 4 head-groups (4 heads each).
Each core computes qkv projection for its heads, attention, and a partial
output projection (its rows of w_proj); the host sums the 4 partials per
batch and adds b_proj.

Device-side design (v2 — software-pipelined):
  - Emission interleaves four streams so no engine idles at phase
    boundaries: projection chunk tc+1 and output-projection chunk ici-1
    are emitted as PE "filler" items inside attention chunk ici (which is
    ACT-exp bound).
  - RoPE: head dims are reordered PAIR-ADJACENT on partitions (host-side
    weight/cos/sin permutation), so rotate-half is a single full-rate DVE
    stream_shuffle; the sin-product runs on GpSimd; only 2 tensor_tensor
    passes remain on the DVE per tile.
  - Causal trim: for diagonal j-blocks only i >= jb*128 columns are
    computed (QK, exp, AV all shrink); the remaining triangle is masked
    by a fixed [128,128] affine_select per diagonal half.
  - Scores are computed transposed (S^T[j,i]) so softmax needs no
    partition reductions: AV with a ones-augmented V (65th row) yields
    denominators, which are broadcast via a tiny K=1 PE matmul and
    inverted with reciprocal_approx_fast.
"""

import sys
import os
from collections import deque

for _p in ("/opt/trn_rl_repo", "/root/.axon_site/_ro/trn_rl_repo"):
    if os.path.isdir(_p) and _p not in sys.path:
        sys.path.insert(0, _p)

import numpy as np
import concourse.bass as bass
import concourse.mybir as mybir
import concourse.tile as tile
from concourse import bacc
from concourse.bass_utils import run_bass_kernel_spmd

B, T, C, H = 2, 2048, 1024, 16
HS = C // H          # 64
HALF = HS // 2       # 32
NCORES = 8
NH = 4               # heads per core
TCH = 512            # t-chunk for projections / i-chunk for attention
CB = C // 128        # 8 contraction blocks
NTB = T // 128       # 16 t/j blocks
NCH = T // TCH       # 4 chunks
F32 = mybir.dt.float32
F32R = mybir.dt.float32r
BF16 = mybir.dt.bfloat16
USE_BF16 = os.environ.get("KERNEL_BF16", "0") == "1"
MMD = BF16 if USE_BF16 else F32R
AF = mybir.ActivationFunctionType
ALU = mybir.AluOpType
SWAP_MASK = [i ^ 1 for i in range(32)]   # pairwise partner within 32-block

_CACHED = {}


def _half_geom(ici, jb):
    """(i-offset within the 512 chunk, width) for j-block jb of chunk ici."""
    r = jb - 4 * ici
    off = max(r, 0) * 128
    return off, TCH - off


def _build_nc():
    nc = bacc.Bacc("TRN2", target_bir_lowering=False, debug=False)

    xt = nc.dram_tensor("xt", [C, T], MMD, kind="ExternalInput").ap()
    wqk = nc.dram_tensor("wqk", [C, 512], MMD, kind="ExternalInput").ap()
    wv = nc.dram_tensor("wv", [C, 256], MMD, kind="ExternalInput").ap()
    wproj = nc.dram_tensor("wproj", [256, C], MMD, kind="ExternalInput").ap()
    bqk = nc.dram_tensor("bqk", [4, 128], F32, kind="ExternalInput").ap()
    cosrep = nc.dram_tensor("cosrep", [128, T], F32, kind="ExternalInput").ap()
    sinsw = nc.dram_tensor("sinsw", [128, T], F32, kind="ExternalInput").ap()
    yout = nc.dram_tensor("yout", [T, C], F32, kind="ExternalOutput").ap()

    with tile.TileContext(nc) as tc:
        with (
            tc.tile_pool(name="const", bufs=1) as const,
            tc.tile_pool(name="persist", bufs=1) as persist,
            tc.tile_pool(name="work", bufs=2) as work,
            tc.tile_pool(name="xtp", bufs=2) as xtp,
            tc.tile_pool(name="attnp", bufs=4) as attnp,
            tc.tile_pool(name="ps", bufs=1, space="PSUM") as ps,
        ):
            # ---- constants -------------------------------------------------
            # DMA issue placement matters: sync carries the startup-critical
            # stream; scalar gets the big weight bodies (it is idle pre-rope);
            # cos/sin are split per-chunk so rope chunk 0 starts early.
            wqk_sb = const.tile([128, CB * 512], MMD)
            nc.sync.dma_start(out=wqk_sb[:, 0:512], in_=wqk[0:128, :])
            xts = {}
            cos_sb = const.tile([128, T], F32)
            sin_sb = const.tile([128, T], F32)

            def load_xt(tcid):
                if tcid >= NCH:
                    return
                t = xtp.tile([128, CB * TCH], MMD, tag="xt", bufs=2, name=f"xt{tcid}")
                tv = t.rearrange("p (cb t) -> p cb t", cb=CB)
                src = xt.rearrange("(cb p) t -> p cb t", p=128)[
                    :, :, tcid * TCH : (tcid + 1) * TCH
                ]
                if tcid == 0:
                    nc.scalar.dma_start(out=tv[:, 0:2], in_=src[:, 0:2])
                    nc.scalar.dma_start(out=tv[:, 2:], in_=src[:, 2:])
                else:
                    nc.sync.dma_start(out=tv, in_=src)
                xts[tcid] = t

            load_xt(0)
            # sync queue continues with the weight body (parallel with xt0 on
            # the scalar queue so the first qk accumulation unblocks ~2x sooner)
            nc.sync.dma_start(
                out=wqk_sb[:, 512:].rearrange("p (cb m) -> p cb m", cb=CB - 1),
                in_=wqk[128:, :].rearrange("(cb p) m -> p cb m", p=128),
            )
            # gpsimd queue: rope constants (gpsimd has no early compute)
            bqk_sb = const.tile([128, 4], F32)
            for mt in range(4):
                nc.gpsimd.dma_start(out=bqk_sb[:, mt : mt + 1], in_=bqk[mt, :][:, None])
            nc.gpsimd.dma_start(out=cos_sb[:, 0:TCH], in_=cosrep[:, 0:TCH])
            nc.gpsimd.dma_start(out=sin_sb[:, 0:TCH], in_=sinsw[:, 0:TCH])
            wv_sb = const.tile([128, CB * 256], MMD)
            nc.scalar.dma_start(
                out=wv_sb.rearrange("p (cb m) -> p cb m", cb=CB),
                in_=wv.rearrange("(cb p) m -> p cb m", p=128),
            )
            nc.gpsimd.dma_start(out=cos_sb[:, TCH:], in_=cosrep[:, TCH:])
            nc.gpsimd.dma_start(out=sin_sb[:, TCH:], in_=sinsw[:, TCH:])
            wproj_sb = const.tile([128, 2 * C], MMD)
            scr = const.tile([128, 1], F32)
            nc.gpsimd.memset(scr, 0.0)
            # warm the exp table set while DMAs run
            nc.scalar.activation(scr, scr, AF.Exp, scale=1.0)

            # ---- persistent intermediates ----------------------------------
            qt_sb = persist.tile([128, 2 * T], MMD)   # [Q01 | Q23], pair-adj d
            kt_sb = persist.tile([128, 2 * T], MMD)
            v_sb = persist.tile([128, NTB * 260], MMD)  # per jb: 4x(64 v + 1 one)
            ctx0 = persist.tile([128, T], MMD)        # heads 0,1 ctxT (normalized)
            ctx1 = persist.tile([128, T], MMD)
            ones_sb = const.tile([128, 64], F32)
            nc.gpsimd.memset(ones_sb, 1.0)
            nc.vector.tensor_copy(
                v_sb.rearrange("p (tb h d) -> p tb h d", tb=NTB, h=4)[:, :, :, 64:65],
                ones_sb.rearrange("p (a b c) -> p a b c", a=NTB, b=4),
            )

            # ---- projection chunk items ------------------------------------
            def qk_item(tcid, mt):
                def go():
                    xt_t = xts[tcid]
                    pq = ps.tile([128, 512], F32, tag="pa", bufs=2, name=f"pq{tcid}_{mt}")
                    for cb in range(CB):
                        nc.tensor.matmul(
                            pq,
                            lhsT=wqk_sb[:, cb * 512 + mt * 128 : cb * 512 + (mt + 1) * 128],
                            rhs=xt_t[:, cb * TCH : (cb + 1) * TCH],
                            start=(cb == 0),
                            stop=(cb == CB - 1),
                        )
                    qb = work.tile([128, 512], F32, tag="qb", bufs=2, name=f"qb{tcid}_{mt}")
                    nc.scalar.activation(
                        qb, pq, AF.Identity, bias=bqk_sb[:, mt : mt + 1], scale=1.0
                    )
                    qsw = work.tile([128, 512], F32, tag="qsw", bufs=2, name=f"qsw{tcid}_{mt}")
                    nc.vector.stream_shuffle(qsw, qb, SWAP_MASK)
                    tsl = slice(tcid * TCH, (tcid + 1) * TCH)
                    t1 = work.tile([128, 512], F32, tag="t1", bufs=2, name=f"t1_{tcid}_{mt}")
                    nc.vector.tensor_mul(t1, qsw, sin_sb[:, tsl])
                    m1 = work.tile([128, 512], F32, tag="m1", bufs=2, name=f"m1_{tcid}_{mt}")
                    nc.vector.tensor_mul(m1, qb, cos_sb[:, tsl])
                    dest = qt_sb if mt < 2 else kt_sb
                    dcol = (mt % 2) * T + tcid * TCH
                    nc.vector.tensor_add(dest[:, dcol : dcol + 512], m1, t1)
                return go

            def v_item(tcid, half):
                def go():
                    xt_t = xts[tcid]
                    pv = ps.tile([128, 512], F32, tag="pa", bufs=2, name=f"pv{tcid}_{half}")
                    for sub in range(2):
                        tl = half * 2 + sub
                        for cb in range(CB):
                            nc.tensor.matmul(
                                pv[:, sub * 256 : (sub + 1) * 256],
                                lhsT=xt_t[:, cb * TCH + tl * 128 : cb * TCH + (tl + 1) * 128],
                                rhs=wv_sb[:, cb * 256 : (cb + 1) * 256],
                                start=(cb == 0),
                                stop=(cb == CB - 1),
                            )
                    tb0 = tcid * 4 + half * 2
                    nc.scalar.copy(
                        v_sb[:, tb0 * 260 : (tb0 + 2) * 260].rearrange(
                            "p (tb h d) -> p tb h d", tb=2, h=4
                        )[:, :, :, 0:64],
                        pv.rearrange("p (tb h d) -> p tb h d", tb=2, h=4),
                    )
                return go

            def proj_items(tcid):
                # mt order 0,2,1,3: pair0's q/k rope completes first so the
                # next attention chunk's first duos unblock earlier
                items = [lambda t=tcid: load_xt(t + 1)]
                items += [qk_item(tcid, mt) for mt in (0, 2, 1, 3)]
                items += [v_item(tcid, h) for h in (0, 1)]
                return items

            # ---- output-projection items (one psum bank each) ---------------
            def out_item(ici, tl, ncol):
                def go():
                    tb = ici * 4 + tl
                    yp = ps.tile([128, 512], F32, tag="pa", bufs=2, name=f"yp{tb}_{ncol}")
                    for cb in range(2):
                        ctx_t = ctx0 if cb == 0 else ctx1
                        nc.tensor.matmul(
                            yp,
                            lhsT=ctx_t[:, tb * 128 : (tb + 1) * 128],
                            rhs=wproj_sb[:, cb * C + ncol * 512 : cb * C + (ncol + 1) * 512],
                            start=(cb == 0),
                            stop=(cb == 1),
                        )
                    ysb = work.tile([128, 512], F32, tag="ysb", bufs=2, name=f"ysb{tb}_{ncol}")
                    nc.vector.tensor_copy(ysb, yp)
                    nc.sync.dma_start(
                        out=yout[tb * 128 : (tb + 1) * 128, ncol * 512 : (ncol + 1) * 512],
                        in_=ysb,
                    )
                return go

            def out_items(ici):
                return [out_item(ici, tl, ncol) for tl in range(4) for ncol in range(2)]

            # ---- attention chunk -------------------------------------------
            def attn_chunk(ici, fillers):
                njb = 4 * (ici + 1)
                i0 = ici * TCH
                for pair in range(2):
                    qt_p = qt_sb[:, pair * T : (pair + 1) * T]
                    kt_p = kt_sb[:, pair * T : (pair + 1) * T]
                    ctxps = ps.tile([65, 1024], F32, tag="ctx", bufs=1, name=f"cps{ici}_{pair}")
                    pending = []

                    def emit_av(entry):
                        ats, geom = entry
                        for hh in range(2):
                            for (jb, off, w, pk) in geom:
                                h_loc = pair * 2 + hh
                                nc.tensor.matmul(
                                    ctxps[0:65, hh * 512 + off : hh * 512 + off + w],
                                    lhsT=v_sb[:, jb * 260 + h_loc * 65 : jb * 260 + (h_loc + 1) * 65],
                                    rhs=ats[hh][:, pk : pk + w],
                                    start=(jb == 0),
                                    stop=(jb == njb - 1),
                                    skip_group_check=True,
                                )

                    for d in range(njb // 2):
                        if fillers:
                            fillers.popleft()()
                        geom = []
                        pk = 0
                        for jb in (2 * d, 2 * d + 1):
                            off, w = _half_geom(ici, jb)
                            geom.append((jb, off, w, pk))
                            pk += w
                        atw = pk
                        sts = [
                            ps.tile([128, 1024], F32, tag="st", bufs=2, name=f"st{ici}_{pair}_{d}_{hh}")
                            for hh in range(2)
                        ]
                        for (jb, off, w, pkh) in geom:
                            for hh in range(2):
                                nc.tensor.matmul(
                                    sts[hh][:, pkh : pkh + w],
                                    lhsT=kt_p[hh * 64 : (hh + 1) * 64, jb * 128 : (jb + 1) * 128],
                                    rhs=qt_p[hh * 64 : (hh + 1) * 64, i0 + off : i0 + off + w],
                                    start=True,
                                    stop=True,
                                )
                        ats = []
                        for hh in range(2):
                            at = attnp.tile([128, 1024], MMD, tag="attn", bufs=4, name=f"at{ici}_{pair}_{d}_{hh}")
                            nc.scalar.activation(at[:, 0:atw], sts[hh][:, 0:atw], AF.Exp, scale=0.125)
                            for (jb, off, w, pkh) in geom:
                                if jb >= 4 * ici:  # diagonal half: triangle in first 128 cols
                                    nc.gpsimd.affine_select(
                                        out=at[:, pkh : pkh + 128],
                                        in_=at[:, pkh : pkh + 128],
                                        compare_op=ALU.is_ge,
                                        fill=0.0,
                                        base=0,
                                        channel_multiplier=-1,
                                        pattern=[[1, 128]],
                                    )
                            ats.append(at)
                        if pending:
                            emit_av(pending.pop(0))
                        pending.append((ats, geom))
                    while pending:
                        emit_av(pending.pop(0))

                    # normalization: denom row 64 -> PE broadcast -> 1/x -> mul
                    if fillers:
                        fillers.popleft()()
                    dn = work.tile([1, 1024], F32, tag="dn", bufs=2, name=f"dn{ici}_{pair}")
                    nc.vector.tensor_copy(dn, ctxps[64:65, :])
                    rc = work.tile([1, 1024], F32, tag="rc", bufs=2, name=f"rc{ici}_{pair}")
                    nc.vector.reciprocal_approx_fast(out=rc, in_=dn)
                    inv = work.tile([128, 1024], F32, tag="inv", bufs=2, name=f"inv{ici}_{pair}")
                    nc.gpsimd.partition_broadcast(inv, rc)
                    ctx_p = ctx0 if pair == 0 else ctx1
                    for hh in range(2):
                        nc.vector.tensor_mul(
                            ctx_p[hh * 64 : (hh + 1) * 64, i0 : i0 + 512],
                            ctxps[0:64, hh * 512 : (hh + 1) * 512],
                            inv[hh * 64 : (hh + 1) * 64, hh * 512 : (hh + 1) * 512],
                        )

            # ---- schedule ---------------------------------------------------
            for f in proj_items(0):
                f()
            nc.scalar.dma_start(
                out=wproj_sb.rearrange("p (cb n) -> p cb n", cb=2),
                in_=wproj.rearrange("(cb p) n -> p cb n", p=128),
            )
            fillers = deque(proj_items(1))
            attn_chunk(0, fillers)
            while fillers:
                fillers.popleft()()
            fillers = deque(proj_items(2))
            attn_chunk(1, fillers)
            while fillers:
                fillers.popleft()()
            fillers = deque(proj_items(3) + out_items(0))
            attn_chunk(2, fillers)
            while fillers:
                fillers.popleft()()
            fillers = deque(out_items(1) + out_items(2))
            attn_chunk(3, fillers)
            while fillers:
                fillers.popleft()()
            for f in out_items(3):
                f()

    nc.compile()
    return nc


def _prep_core_inputs(x, cos, sin, w_attn, b_attn, w_proj):
    """Build the 8 per-core input maps (host-side shard/reorder)."""
    import ml_dtypes

    mmnp = ml_dtypes.bfloat16 if USE_BF16 else np.float32
    x = np.asarray(x, dtype=np.float32)
    cos = np.asarray(cos, dtype=np.float32).reshape(T, HALF)
    sin = np.asarray(sin, dtype=np.float32).reshape(T, HALF)
    w_attn = np.asarray(w_attn, dtype=np.float32)
    b_attn = np.asarray(b_attn, dtype=np.float32)
    w_proj = np.asarray(w_proj, dtype=np.float32)

    cosT = np.ascontiguousarray(cos.T)               # [32, T]
    sinT = np.ascontiguousarray(sin.T)

    # pair-adjacent d order: partition p (within a head's 64) -> orig d
    p64 = np.arange(64)
    d_of_p = (p64 % 2) * HALF + p64 // 2             # [64]
    pairidx = p64 // 2
    member = p64 % 2
    cos64 = cosT[pairidx]                            # [64, T]
    sin64 = sinT[pairidx] * np.where(member == 0, -1.0, 1.0)[:, None].astype(np.float32)
    cosrep = np.tile(cos64, (2, 1))                  # [128, T]
    sin_sw = np.tile(sin64, (2, 1))

    xts = [np.ascontiguousarray(x[b].T).astype(mmnp) for b in range(B)]  # [C, T]

    in_maps = []
    for core in range(NCORES):
        b = core // 4
        g = core % 4
        heads = [4 * g + i for i in range(NH)]
        # q/k M-tiles: [Q(h0,h1), Q(h2,h3), K(h0,h1), K(h2,h3)], pair-adj cols
        qcols, bq = [], []
        for mt, (base, hs) in enumerate(
            [(0, heads[0:2]), (0, heads[2:4]), (C, heads[0:2]), (C, heads[2:4])]
        ):
            cols = np.concatenate([base + h * HS + d_of_p for h in hs])
            qcols.append(cols)
            bq.append(b_attn[cols])
        wqk_c = np.ascontiguousarray(w_attn[:, np.concatenate(qcols)]).astype(mmnp)
        bqk_c = np.stack(bq)                                            # [4, 128]
        vcols = np.concatenate(
            [np.arange(2 * C + h * HS, 2 * C + (h + 1) * HS) for h in heads]
        )
        wv_c = np.ascontiguousarray(w_attn[:, vcols]).astype(mmnp)
        wproj_c = np.ascontiguousarray(w_proj[g * 256 : (g + 1) * 256, :]).astype(mmnp)
        in_maps.append(
            {
                "xt": xts[b],
                "wqk": wqk_c,
                "wv": wv_c,
                "wproj": wproj_c,
                "bqk": np.ascontiguousarray(bqk_c),
                "cosrep": np.ascontiguousarray(cosrep),
                "sinsw": np.ascontiguousarray(sin_sw),
            }
        )
    return in_maps


def kernel(x, cos, sin, w_attn, b_attn, w_proj, b_proj, _want_trace=False):
    if "nc" not in _CACHED:
        _CACHED["nc"] = _build_nc()
    nc = _CACHED["nc"]
    in_maps = _prep_core_inputs(x, cos, sin, w_attn, b_attn, w_proj)
    res = run_bass_kernel_spmd(
        nc, in_maps, core_ids=list(range(NCORES)), trace=_want_trace
    )
    _CACHED["last_result"] = res
    b_proj = np.asarray(b_proj, dtype=np.float32)
    # v-bias folds out of attention (softmax rows sum to 1): it contributes a
    # constant b_v @ w_proj to every output row, added here with b_proj.
    bv = np.asarray(b_attn, dtype=np.float32)[2 * C : 3 * C]
    bias_full = b_proj + bv @ np.asarray(w_proj, dtype=np.float32)
    out = np.empty((B, T, C), np.float32)
    for b in range(B):
        acc = res.results[b * 4]["yout"].astype(np.float32).copy()
        for g in range(1, 4):
            acc += res.results[b * 4 + g]["yout"]
        out[b] = acc + bias_full[None, :]
    return out


# revision 21
# speedup vs baseline: 1.1116x; 1.0149x over previous
"""Causal self-attention (B=2, T=2048, C=1024, H=16) on 8 TRN2 NeuronCores.

Sharding: 8 cores = 2 batches x 4 head-groups (4 heads each).
Each core computes qkv projection for its heads, attention, and a partial
output projection (its rows of w_proj); the host sums the 4 partials per
batch and adds b_proj.

Device-side design (v2 — software-pipelined):
  - Emission interleaves four streams so no engine idles at phase
    boundaries: projection chunk tc+1 and output-projection chunk ici-1
    are emitted as PE "filler" items inside attention chunk ici (which is
    ACT-exp bound).
  - RoPE: head dims are reordered PAIR-ADJACENT on partitions (host-side
    weight/cos/sin permutation), so rotate-half is a single full-rate DVE
    stream_shuffle; the sin-product runs on GpSimd; only 2 tensor_tensor
    passes remain on the DVE per tile.
  - Causal trim: for diagonal j-blocks only i >= jb*128 columns are
    computed (QK, exp, AV all shrink); the remaining triangle is masked
    by a fixed [128,128] affine_select per diagonal half.
  - Scores are computed transposed (S^T[j,i]) so softmax needs no
    partition reductions: AV with a ones-augmented V (65th row) yields
    denominators, which are broadcast via a tiny K=1 PE matmul and
    inverted with reciprocal_approx_fast.
"""

import sys
import os
from collections import deque

for _p in ("/opt/trn_rl_repo", "/root/.axon_site/_ro/trn_rl_repo"):
    if os.path.isdir(_p) and _p not in sys.path:
        sys.path.insert(0, _p)

import numpy as np
import concourse.bass as bass
import concourse.mybir as mybir
import concourse.tile as tile
from concourse import bacc
from concourse.bass_utils import run_bass_kernel_spmd

B, T, C, H = 2, 2048, 1024, 16
HS = C // H          # 64
HALF = HS // 2       # 32
NCORES = 8
NH = 4               # heads per core
TCH = 512            # t-chunk for projections / i-chunk for attention
CB = C // 128        # 8 contraction blocks
NTB = T // 128       # 16 t/j blocks
NCH = T // TCH       # 4 chunks
F32 = mybir.dt.float32
F32R = mybir.dt.float32r
BF16 = mybir.dt.bfloat16
USE_BF16 = os.environ.get("KERNEL_BF16", "0") == "1"
MMD = BF16 if USE_BF16 else F32R
AF = mybir.ActivationFunctionType
ALU = mybir.AluOpType
SWAP_MASK = [i ^ 1 for i in range(32)]   # pairwise partner within 32-block

_CACHED = {}


def _half_geom(ici, jb):
    """(i-offset within the 512 chunk, width) for j-block jb of chunk ici."""
    r = jb - 4 * ici
    off = max(r, 0) * 128
    return off, TCH - off


def _build_nc():
    nc = bacc.Bacc("TRN2", target_bir_lowering=False, debug=False)

    xt = nc.dram_tensor("xt", [C, T], MMD, kind="ExternalInput").ap()
    wqk = nc.dram_tensor("wqk", [C, 512], MMD, kind="ExternalInput").ap()
    wv = nc.dram_tensor("wv", [C, 256], MMD, kind="ExternalInput").ap()
    wproj = nc.dram_tensor("wproj", [256, C], MMD, kind="ExternalInput").ap()
    bqk = nc.dram_tensor("bqk", [4, 128], F32, kind="ExternalInput").ap()
    cosrep = nc.dram_tensor("cosrep", [128, T], F32, kind="ExternalInput").ap()
    sinsw = nc.dram_tensor("sinsw", [128, T], F32, kind="ExternalInput").ap()
    yout = nc.dram_tensor("yout", [T, C], F32, kind="ExternalOutput").ap()

    with tile.TileContext(nc) as tc:
        with (
            tc.tile_pool(name="const", bufs=1) as const,
            tc.tile_pool(name="persist", bufs=1) as persist,
            tc.tile_pool(name="work", bufs=2) as work,
            tc.tile_pool(name="xtp", bufs=2) as xtp,
            tc.tile_pool(name="attnp", bufs=4) as attnp,
            tc.tile_pool(name="ps", bufs=1, space="PSUM") as ps,
        ):
            # ---- constants -------------------------------------------------
            # DMA issue placement matters: sync carries the startup-critical
            # stream; scalar gets the big weight bodies (it is idle pre-rope);
            # cos/sin are split per-chunk so rope chunk 0 starts early.
            wqk_sb = const.tile([128, CB * 512], MMD)
            nc.sync.dma_start(out=wqk_sb[:, 0:512], in_=wqk[0:128, :])
            xts = {}
            cos_sb = const.tile([128, T], F32)
            sin_sb = const.tile([128, T], F32)

            def load_xt(tcid):
                if tcid >= NCH:
                    return
                t = xtp.tile([128, CB * TCH], MMD, tag="xt", bufs=2, name=f"xt{tcid}")
                tv = t.rearrange("p (cb t) -> p cb t", cb=CB)
                src = xt.rearrange("(cb p) t -> p cb t", p=128)[
                    :, :, tcid * TCH : (tcid + 1) * TCH
                ]
                if tcid == 0:
                    nc.sync.dma_start(out=tv[:, 0:2], in_=src[:, 0:2])
                    nc.sync.dma_start(out=tv[:, 2:], in_=src[:, 2:])
                else:
                    nc.sync.dma_start(out=tv, in_=src)
                xts[tcid] = t

            load_xt(0)
            # sync queue carries ALL big transfers in PE-dependency order (a
            # large dma_start can block its issuing engine for >10us, so the
            # scalar engine must issue nothing)
            nc.sync.dma_start(
                out=wqk_sb[:, 512:].rearrange("p (cb m) -> p cb m", cb=CB - 1),
                in_=wqk[128:, :].rearrange("(cb p) m -> p cb m", p=128),
            )
            wv_sb = const.tile([128, CB * 256], MMD)
            nc.sync.dma_start(
                out=wv_sb.rearrange("p (cb m) -> p cb m", cb=CB),
                in_=wv.rearrange("(cb p) m -> p cb m", p=128),
            )
            # gpsimd queue: rope constants (gpsimd has no early compute)
            bqk_sb = const.tile([128, 4], F32)
            for mt in range(4):
                nc.gpsimd.dma_start(out=bqk_sb[:, mt : mt + 1], in_=bqk[mt, :][:, None])
            nc.gpsimd.dma_start(out=cos_sb[:, 0:TCH], in_=cosrep[:, 0:TCH])
            nc.gpsimd.dma_start(out=sin_sb[:, 0:TCH], in_=sinsw[:, 0:TCH])
            nc.gpsimd.dma_start(out=cos_sb[:, TCH:], in_=cosrep[:, TCH:])
            nc.gpsimd.dma_start(out=sin_sb[:, TCH:], in_=sinsw[:, TCH:])
            wproj_sb = const.tile([128, 2 * C], MMD)
            scr = const.tile([128, 1], F32)
            nc.gpsimd.memset(scr, 0.0)
            # warm the exp table set while DMAs run
            nc.scalar.activation(scr, scr, AF.Exp, scale=1.0)

            # ---- persistent intermediates ----------------------------------
            qt_sb = persist.tile([128, 2 * T], MMD)   # [Q01 | Q23], pair-adj d
            kt_sb = persist.tile([128, 2 * T], MMD)
            v_sb = persist.tile([128, NTB * 260], MMD)  # per jb: 4x(64 v + 1 one)
            ctx0 = persist.tile([128, T], MMD)        # heads 0,1 ctxT (normalized)
            ctx1 = persist.tile([128, T], MMD)
            ones_sb = const.tile([128, 64], F32)
            nc.gpsimd.memset(ones_sb, 1.0)
            nc.vector.tensor_copy(
                v_sb.rearrange("p (tb h d) -> p tb h d", tb=NTB, h=4)[:, :, :, 64:65],
                ones_sb.rearrange("p (a b c) -> p a b c", a=NTB, b=4),
            )

            # ---- projection chunk items ------------------------------------
            def qk_item(tcid, mt):
                def go():
                    xt_t = xts[tcid]
                    pq = ps.tile([128, 512], F32, tag="pa", bufs=2, name=f"pq{tcid}_{mt}")
                    for cb in range(CB):
                        nc.tensor.matmul(
                            pq,
                            lhsT=wqk_sb[:, cb * 512 + mt * 128 : cb * 512 + (mt + 1) * 128],
                            rhs=xt_t[:, cb * TCH : (cb + 1) * TCH],
                            start=(cb == 0),
                            stop=(cb == CB - 1),
                        )
                    qb = work.tile([128, 512], F32, tag="qb", bufs=2, name=f"qb{tcid}_{mt}")
                    nc.scalar.activation(
                        qb, pq, AF.Identity, bias=bqk_sb[:, mt : mt + 1], scale=1.0
                    )
                    qsw = work.tile([128, 512], F32, tag="qsw", bufs=2, name=f"qsw{tcid}_{mt}")
                    nc.vector.stream_shuffle(qsw, qb, SWAP_MASK)
                    tsl = slice(tcid * TCH, (tcid + 1) * TCH)
                    t1 = work.tile([128, 512], F32, tag="t1", bufs=2, name=f"t1_{tcid}_{mt}")
                    nc.vector.tensor_mul(t1, qsw, sin_sb[:, tsl])
                    m1 = work.tile([128, 512], F32, tag="m1", bufs=2, name=f"m1_{tcid}_{mt}")
                    nc.vector.tensor_mul(m1, qb, cos_sb[:, tsl])
                    dest = qt_sb if mt < 2 else kt_sb
                    dcol = (mt % 2) * T + tcid * TCH
                    nc.vector.tensor_add(dest[:, dcol : dcol + 512], m1, t1)
                return go

            def v_item(tcid, half):
                def go():
                    xt_t = xts[tcid]
                    pv = ps.tile([128, 512], F32, tag="pa", bufs=2, name=f"pv{tcid}_{half}")
                    for sub in range(2):
                        tl = half * 2 + sub
                        for cb in range(CB):
                            nc.tensor.matmul(
                                pv[:, sub * 256 : (sub + 1) * 256],
                                lhsT=xt_t[:, cb * TCH + tl * 128 : cb * TCH + (tl + 1) * 128],
                                rhs=wv_sb[:, cb * 256 : (cb + 1) * 256],
                                start=(cb == 0),
                                stop=(cb == CB - 1),
                            )
                    tb0 = tcid * 4 + half * 2
                    nc.scalar.copy(
                        v_sb[:, tb0 * 260 : (tb0 + 2) * 260].rearrange(
                            "p (tb h d) -> p tb h d", tb=2, h=4
                        )[:, :, :, 0:64],
                        pv.rearrange("p (tb h d) -> p tb h d", tb=2, h=4),
                    )
                return go

            def proj_items(tcid):
                # mt order 0,2,1,3: pair0's q/k rope completes first so the
                # next attention chunk's first duos unblock earlier
                items = [lambda t=tcid: load_xt(t + 1)]
                items += [qk_item(tcid, mt) for mt in (0, 2, 1, 3)]
                items += [v_item(tcid, h) for h in (0, 1)]
                return items

            # ---- output-projection items (one psum bank each) ---------------
            def out_item(ici, tl, ncol):
                def go():
                    tb = ici * 4 + tl
                    yp = ps.tile([128, 512], F32, tag="pa", bufs=2, name=f"yp{tb}_{ncol}")
                    for cb in range(2):
                        ctx_t = ctx0 if cb == 0 else ctx1
                        nc.tensor.matmul(
                            yp,
                            lhsT=ctx_t[:, tb * 128 : (tb + 1) * 128],
                            rhs=wproj_sb[:, cb * C + ncol * 512 : cb * C + (ncol + 1) * 512],
                            start=(cb == 0),
                            stop=(cb == 1),
                        )
                    ysb = work.tile([128, 512], F32, tag="ysb", bufs=2, name=f"ysb{tb}_{ncol}")
                    nc.vector.tensor_copy(ysb, yp)
                    nc.sync.dma_start(
                        out=yout[tb * 128 : (tb + 1) * 128, ncol * 512 : (ncol + 1) * 512],
                        in_=ysb,
                    )
                return go

            def out_items(ici):
                return [out_item(ici, tl, ncol) for tl in range(4) for ncol in range(2)]

            # ---- attention chunk -------------------------------------------
            def attn_chunk(ici, fillers):
                njb = 4 * (ici + 1)
                i0 = ici * TCH
                for pair in range(2):
                    qt_p = qt_sb[:, pair * T : (pair + 1) * T]
                    kt_p = kt_sb[:, pair * T : (pair + 1) * T]
                    ctxps = ps.tile([65, 1024], F32, tag="ctx", bufs=1, name=f"cps{ici}_{pair}")
                    pending = []

                    def emit_av(entry):
                        ats, geom = entry
                        for hh in range(2):
                            for (jb, off, w, pk) in geom:
                                h_loc = pair * 2 + hh
                                nc.tensor.matmul(
                                    ctxps[0:65, hh * 512 + off : hh * 512 + off + w],
                                    lhsT=v_sb[:, jb * 260 + h_loc * 65 : jb * 260 + (h_loc + 1) * 65],
                                    rhs=ats[hh][:, pk : pk + w],
                                    start=(jb == 0),
                                    stop=(jb == njb - 1),
                                    skip_group_check=True,
                                )

                    for d in range(njb // 2):
                        if fillers:
                            fillers.popleft()()
                        geom = []
                        pk = 0
                        for jb in (2 * d, 2 * d + 1):
                            off, w = _half_geom(ici, jb)
                            geom.append((jb, off, w, pk))
                            pk += w
                        atw = pk
                        sts = [
                            ps.tile([128, 1024], F32, tag="st", bufs=2, name=f"st{ici}_{pair}_{d}_{hh}")
                            for hh in range(2)
                        ]
                        for (jb, off, w, pkh) in geom:
                            for hh in range(2):
                                nc.tensor.matmul(
                                    sts[hh][:, pkh : pkh + w],
                                    lhsT=kt_p[hh * 64 : (hh + 1) * 64, jb * 128 : (jb + 1) * 128],
                                    rhs=qt_p[hh * 64 : (hh + 1) * 64, i0 + off : i0 + off + w],
                                    start=True,
                                    stop=True,
                                )
                        ats = []
                        for hh in range(2):
                            at = attnp.tile([128, 1024], MMD, tag="attn", bufs=4, name=f"at{ici}_{pair}_{d}_{hh}")
                            nc.scalar.activation(at[:, 0:atw], sts[hh][:, 0:atw], AF.Exp, scale=0.125)
                            for (jb, off, w, pkh) in geom:
                                if jb >= 4 * ici:  # diagonal half: triangle in first 128 cols
                                    nc.gpsimd.affine_select(
                                        out=at[:, pkh : pkh + 128],
                                        in_=at[:, pkh : pkh + 128],
                                        compare_op=ALU.is_ge,
                                        fill=0.0,
                                        base=0,
                                        channel_multiplier=-1,
                                        pattern=[[1, 128]],
                                    )
                            ats.append(at)
                        if pending:
                            emit_av(pending.pop(0))
                        pending.append((ats, geom))
                    while pending:
                        emit_av(pending.pop(0))

                    # normalization: denom row 64 -> PE broadcast -> 1/x -> mul
                    if fillers:
                        fillers.popleft()()
                    dn = work.tile([1, 1024], F32, tag="dn", bufs=2, name=f"dn{ici}_{pair}")
                    nc.vector.tensor_copy(dn, ctxps[64:65, :])
                    rc = work.tile([1, 1024], F32, tag="rc", bufs=2, name=f"rc{ici}_{pair}")
                    nc.vector.reciprocal_approx_fast(out=rc, in_=dn)
                    inv = work.tile([128, 1024], F32, tag="inv", bufs=2, name=f"inv{ici}_{pair}")
                    nc.gpsimd.partition_broadcast(inv, rc)
                    ctx_p = ctx0 if pair == 0 else ctx1
                    for hh in range(2):
                        nc.vector.tensor_mul(
                            ctx_p[hh * 64 : (hh + 1) * 64, i0 : i0 + 512],
                            ctxps[0:64, hh * 512 : (hh + 1) * 512],
                            inv[hh * 64 : (hh + 1) * 64, hh * 512 : (hh + 1) * 512],
                        )

            # ---- schedule ---------------------------------------------------
            for f in proj_items(0):
                f()
            nc.sync.dma_start(
                out=wproj_sb.rearrange("p (cb n) -> p cb n", cb=2),
                in_=wproj.rearrange("(cb p) n -> p cb n", p=128),
            )
            fillers = deque(proj_items(1))
            attn_chunk(0, fillers)
            while fillers:
                fillers.popleft()()
            fillers = deque(proj_items(2))
            attn_chunk(1, fillers)
            while fillers:
                fillers.popleft()()
            fillers = deque(proj_items(3) + out_items(0))
            attn_chunk(2, fillers)
            while fillers:
                fillers.popleft()()
            fillers = deque(out_items(1) + out_items(2))
            attn_chunk(3, fillers)
            while fillers:
                fillers.popleft()()
            for f in out_items(3):
                f()

    nc.compile()
    return nc


def _prep_core_inputs(x, cos, sin, w_attn, b_attn, w_proj):
    """Build the 8 per-core input maps (host-side shard/reorder)."""
    import ml_dtypes

    mmnp = ml_dtypes.bfloat16 if USE_BF16 else np.float32
    x = np.asarray(x, dtype=np.float32)
    cos = np.asarray(cos, dtype=np.float32).reshape(T, HALF)
    sin = np.asarray(sin, dtype=np.float32).reshape(T, HALF)
    w_attn = np.asarray(w_attn, dtype=np.float32)
    b_attn = np.asarray(b_attn, dtype=np.float32)
    w_proj = np.asarray(w_proj, dtype=np.float32)

    cosT = np.ascontiguousarray(cos.T)               # [32, T]
    sinT = np.ascontiguousarray(sin.T)

    # pair-adjacent d order: partition p (within a head's 64) -> orig d
    p64 = np.arange(64)
    d_of_p = (p64 % 2) * HALF + p64 // 2             # [64]
    pairidx = p64 // 2
    member = p64 % 2
    cos64 = cosT[pairidx]                            # [64, T]
    sin64 = sinT[pairidx] * np.where(member == 0, -1.0, 1.0)[:, None].astype(np.float32)
    cosrep = np.tile(cos64, (2, 1))                  # [128, T]
    sin_sw = np.tile(sin64, (2, 1))

    xts = [np.ascontiguousarray(x[b].T).astype(mmnp) for b in range(B)]  # [C, T]

    in_maps = []
    for core in range(NCORES):
        b = core // 4
        g = core % 4
        heads = [4 * g + i for i in range(NH)]
        # q/k M-tiles: [Q(h0,h1), Q(h2,h3), K(h0,h1), K(h2,h3)], pair-adj cols
        qcols, bq = [], []
        for mt, (base, hs) in enumerate(
            [(0, heads[0:2]), (0, heads[2:4]), (C, heads[0:2]), (C, heads[2:4])]
        ):
            cols = np.concatenate([base + h * HS + d_of_p for h in hs])
            qcols.append(cols)
            bq.append(b_attn[cols])
        wqk_c = np.ascontiguousarray(w_attn[:, np.concatenate(qcols)]).astype(mmnp)
        bqk_c = np.stack(bq)                                            # [4, 128]
        vcols = np.concatenate(
            [np.arange(2 * C + h * HS, 2 * C + (h + 1) * HS) for h in heads]
        )
        wv_c = np.ascontiguousarray(w_attn[:, vcols]).astype(mmnp)
        wproj_c = np.ascontiguousarray(w_proj[g * 256 : (g + 1) * 256, :]).astype(mmnp)
        in_maps.append(
            {
                "xt": xts[b],
                "wqk": wqk_c,
                "wv": wv_c,
                "wproj": wproj_c,
                "bqk": np.ascontiguousarray(bqk_c),
                "cosrep": np.ascontiguousarray(cosrep),
                "sinsw": np.ascontiguousarray(sin_sw),
            }
        )
    return in_maps


def kernel(x, cos, sin, w_attn, b_attn, w_proj, b_proj, _want_trace=False):
    if "nc" not in _CACHED:
        _CACHED["nc"] = _build_nc()
    nc = _CACHED["nc"]
    in_maps = _prep_core_inputs(x, cos, sin, w_attn, b_attn, w_proj)
    res = run_bass_kernel_spmd(
        nc, in_maps, core_ids=list(range(NCORES)), trace=_want_trace
    )
    _CACHED["last_result"] = res
    b_proj = np.asarray(b_proj, dtype=np.float32)
    # v-bias folds out of attention (softmax rows sum to 1): it contributes a
    # constant b_v @ w_proj to every output row, added here with b_proj.
    bv = np.asarray(b_attn, dtype=np.float32)[2 * C : 3 * C]
    bias_full = b_proj + bv @ np.asarray(w_proj, dtype=np.float32)
    out = np.empty((B, T, C), np.float32)
    for b in range(B):
        acc = res.results[b * 4]["yout"].astype(np.float32).copy()
        for g in range(1, 4):
            acc += res.results[b * 4 + g]["yout"]
        out[b] = acc + bias_full[None, :]
    return out
